# revision 14
# baseline (speedup 1.0000x reference)
"""Trainium2 Bass kernel for NeighborStatOP (retrieval_knn).

Computes, for each frame and each local atom i:
  min_rr2[f, i]  = min_{j != i} |x_j - x_i|^2                      (f32)
  max_nnei[f, t] = max_i #{ j != i : |x_j - x_i|^2 < 6^2, type_j = t } (int32)

Strategy (8 NeuronCores, SPMD, one compiled program):
  - Atoms are sorted host-side by a kd-tree (median splits), so each
    128-query tile occupies a compact box.  For every tile the host builds
    a candidate key list: all atoms within RCUT(+eps) of the tile's box
    (exact point-to-box distance).  Only ~400-1300 of the 4096 keys
    survive -> ~3x less on-chip reduction work.  Candidates are grouped by
    atom type into four fixed-width segments per processing slot; each
    core processes its tiles largest-first and slot widths are the
    per-slot maxima over cores, so a single sparse-region outlier tile
    doesn't inflate every slot.  Padding uses far-away dummy atoms.
  - Core c handles frame c//4, query tiles (c%4)*8 .. +8 (kd order).
  - rr2 via tensor-engine matmul: centered per-component expansion
    qc^2 - 2 qc kc + kc^2 with every fp32 feature split into 3 bf16 pieces
    (bf16 x bf16 products are exact in fp32; PSUM accumulates fp32).
    K = 36 rows, full-rate bf16 matmuls.
  - Self-pair knockout: within each type segment the tile's own atoms are
    listed first, so all BIG (1e30) entries live in a 128-wide window at
    each segment start; a small extra matmul per window accumulates the
    per-tile diagmask (BIG at each query's own column) through
    lhsT = identity, start=False.  Position is data, not code.
  - min: DVE tensor_reduce(min) per slot; counts: per type segment a
    ScalarE Sign pass (accum_out; count = (S + W)/2; self/pads give -1 and
    drop out) or a DVE is_lt pass (count = S) - the segment->engine map
    balances ACT and DVE.
  - Host finalizes: count formula, inverse permutations, max; any query
    whose candidate min is >= RCUT^2 (nearest neighbour outside the
    candidate radius; does not occur at realistic densities) is recomputed
    exactly on host.
"""
import sys

sys.path.insert(0, "/opt/trn_rl_repo")

import numpy as np
import ml_dtypes

NFRAMES = 2
NLOC = 4096
NTYPES = 4
RCUT = 6.0
RCUT2 = 36.0
CENTER = 20.0
BIG = 1.0e30
PAD_COORD = 1000.0
NCORES = 8
CPF = 4                       # cores per frame
QPC = NLOC // CPF             # queries per core = 1024
NQT = QPC // 128              # query tiles (slots) per core = 8
NTILES = NLOC // 128          # query tiles per frame = 32
PAIRS = [(0, 0), (0, 1), (1, 0), (1, 1), (0, 2), (2, 0)]
K = 36                        # 12 split rows per component


def seg_engine(t, tau):
    """Engine for the count pass of slot t, type-segment tau (balance)."""
    return "D" if tau == 3 or (tau == 2 and t >= 4) else "A"

_CACHE = {}


def _split3(x):
    p1 = x.astype(ml_dtypes.bfloat16)
    r1 = (x.astype(np.float64) - p1.astype(np.float64)).astype(np.float32)
    p2 = r1.astype(ml_dtypes.bfloat16)
    r2 = (r1.astype(np.float64) - p2.astype(np.float64)).astype(np.float32)
    p3 = r2.astype(ml_dtypes.bfloat16)
    return [p1, p2, p3]


def _features(coords):
    """coords: (n, 3) float64 centered.
    Returns (qfeat (K, n), kfeat (K, n)) as bfloat16 arrays."""
    n = len(coords)
    ones = np.ones(n, ml_dtypes.bfloat16)
    L, R = [], []
    for d in range(3):
        qc = coords[:, d]
        q2 = (qc * qc).astype(np.float32)
        q2p = _split3(q2)
        qp = _split3(qc.astype(np.float32))
        m2p = _split3((-2.0 * qc).astype(np.float32))
        for i in range(3):              # qc^2 pieces x 1
            L.append(q2p[i]); R.append(ones)
        for (ia, ib) in PAIRS:          # qc pieces x -2kc pieces
            L.append(qp[ia]); R.append(m2p[ib])
        for i in range(3):              # 1 x kc^2 pieces
            L.append(ones); R.append(q2p[i])
    return (np.stack(L).astype(ml_dtypes.bfloat16),
            np.stack(R).astype(ml_dtypes.bfloat16))


def _kd_perm(c):
    """c: (n, 3) raw coords. Recursive median split into 128-atom leaves."""
    def split(ids):
        if len(ids) <= 128:
            return [ids]
        spread = c[ids].max(0) - c[ids].min(0)
        ax = int(np.argmax(spread))
        order = ids[np.argsort(c[ids, ax], kind="stable")]
        h = len(ids) // 2
        return split(order[:h]) + split(order[h:])
    return np.concatenate(split(np.arange(len(c))))


def _build(wslots, repeat=1):
    """Build + lower the SPMD kernel. wslots: per-slot segment widths."""
    import concourse.bacc as bacc
    import concourse.tile as tile
    from concourse import mybir

    f32 = mybir.dt.float32
    bf16 = mybir.dt.bfloat16
    Ws = [4 * w for w in wslots]
    col0 = np.concatenate([[0], np.cumsum(Ws)])      # kfeat col offset per slot
    WTOT = int(col0[-1])
    WBMAX = ((max(Ws) + 511) // 512) * 512

    nc = bacc.Bacc("TRN2", target_bir_lowering=False, debug=False,
                   num_devices=NCORES)
    qf = nc.dram_tensor("qfeat", [K, QPC], bf16, kind="ExternalInput").ap()
    kf = nc.dram_tensor("kfeat", [K, WTOT], bf16, kind="ExternalInput").ap()
    sp = nc.dram_tensor("selfpos", [128, NQT], f32,
                        kind="ExternalInput").ap()
    idn = nc.dram_tensor("ident", [128, 128], bf16, kind="ExternalInput").ap()
    out_min = nc.dram_tensor("out_min", [128, NQT], f32,
                             kind="ExternalOutput").ap()
    out_cnt = nc.dram_tensor("out_cnt", [128, NQT * NTYPES], f32,
                             kind="ExternalOutput").ap()

    WBMAX = ((max(4 * w for w in wslots) + 511) // 512) * 512

    # per-slot PSUM tag assignment: dedicated banks per concurrent slot,
    # narrow slots alternate over two buffers; total must fit 8 banks
    WBs = [((4 * w + 511) // 512) * 512 for w in wslots]
    tags, tag_width = [], {}
    budget = 4096
    for t in range(NQT):
        if WBs[t] > 512:
            tag = f"s{t}"
            tags.append(tag)
            tag_width[tag] = WBs[t]
        else:
            tag = f"n{t % 2}"
            tags.append(tag)
            tag_width[tag] = 512
    tot = sum(tag_width.values())
    if tot > 4096:
        # fall back: collapse wide tags pairwise until it fits
        wide_tags = sorted((tg for tg in tag_width if tg.startswith("s")),
                           key=lambda tg: -tag_width[tg])
        i = 1
        while tot > 4096 and i < len(wide_tags):
            victim = wide_tags[i]
            keeper = wide_tags[i - 1]
            for t in range(NQT):
                if tags[t] == victim:
                    tags[t] = keeper
            tot -= tag_width.pop(victim)
            i += 1

    with tile.TileContext(nc) as tc:
        with (
            tc.tile_pool(name="singles", bufs=1) as singles,
            tc.tile_pool(name="psum", bufs=1, space="PSUM") as psum_pool,
            tc.tile_pool(name="dmp", bufs=3) as dm_pool,
            tc.tile_pool(name="sc_act", bufs=3) as sc_act_pool,
            tc.tile_pool(name="sc_dve", bufs=2) as sc_dve_pool,
        ):
            qsb = singles.tile([K, QPC], bf16)
            nc.sync.dma_start(out=qsb[:], in_=qf)
            ksb = singles.tile([K, WTOT], bf16)
            for t in range(NQT):       # per-slot DMAs so slot 0 starts early
                a, b = int(col0[t]), int(col0[t + 1])
                nc.sync.dma_start(out=ksb[:, a:b], in_=kf[:, a:b])
            spsb = singles.tile([128, NQT], f32)
            nc.sync.dma_start(out=spsb[:], in_=sp)
            idsb = singles.tile([128, 128], bf16)
            nc.sync.dma_start(out=idsb[:], in_=idn)
            bias36 = singles.tile([128, 1], f32)
            nc.vector.memset(bias36[:], RCUT2)
            iot = singles.tile([128, NTYPES * 128], mybir.dt.int32)
            nc.gpsimd.iota(iot[:], pattern=[[1, NTYPES * 128]], base=0,
                           channel_multiplier=0)
            min_sb = singles.tile([128, NQT], f32)
            cnt_sb = singles.tile([128, NQT * NTYPES], f32)

            def do_slot(t):
                w = wslots[t]
                W = 4 * w
                k0 = int(col0[t])
                ps = psum_pool.tile([128, tag_width[tags[t]]], f32,
                                    tag=tags[t])
                # diagmask for this slot, built on GPSIMD:
                # dm[p, c] = BIG iff c == selfpos[p, t]
                dmt = dm_pool.tile([128, NTYPES * 128], bf16, tag="dm")
                nc.gpsimd.tensor_scalar(
                    out=dmt[:], in0=iot[:],
                    scalar1=spsb[:, t:t + 1], scalar2=float(BIG),
                    op0=mybir.AluOpType.is_equal,
                    op1=mybir.AluOpType.mult,
                )
                # diag windows: ps range [tau*w, tau*w+min(128,w)) <- dm col
                # tau*128; split at banks, coalesced when contiguous
                pieces = []
                for tau in range(NTYPES):
                    a = tau * w
                    b = a + min(128, w)
                    dc = tau * 128
                    while a < b:
                        e = min(b, ((a // 512) + 1) * 512)
                        pieces.append([a, e, dc])
                        dc += e - a
                        a = e
                merged = [pieces[0]]
                for a, e, dc in pieces[1:]:
                    pa, pe, pdc = merged[-1]
                    if (a == pe and dc == pdc + (pe - pa)
                            and (a % 512) != 0):
                        merged[-1][1] = e
                    else:
                        merged.append([a, e, dc])
                by_bank = {}
                for a, e, dc in merged:
                    by_bank.setdefault(a // 512, []).append((a, e, dc))
                for b0 in range(0, W, 512):
                    b1 = min(b0 + 512, W)
                    nc.tensor.matmul(
                        ps[:, b0:b1],
                        lhsT=qsb[:, t * 128:(t + 1) * 128],
                        rhs=ksb[:, k0 + b0:k0 + b1],
                        start=True, stop=(b0 // 512) not in by_bank,
                    )
                for bank in sorted(by_bank):
                    subs = by_bank[bank]
                    for i, (a, e, dcol) in enumerate(subs):
                        nc.tensor.matmul(
                            ps[:, a:e],
                            lhsT=idsb[:],
                            rhs=dmt[:, dcol:dcol + (e - a)],
                            start=False, stop=(i == len(subs) - 1),
                            skip_group_check=True,
                        )
                nc.vector.tensor_reduce(
                    out=min_sb[:, t:t + 1], in_=ps[:, 0:W],
                    axis=mybir.AxisListType.X, op=mybir.AluOpType.min,
                )
                for s in range(NTYPES):
                    a, b_ = s * w, (s + 1) * w
                    ccol = t * NTYPES + s
                    if seg_engine(t, s) == "A":
                        sc = sc_act_pool.tile([128, WBMAX], f32, tag="sa")
                        nc.scalar.activation(
                            sc[:, a:b_], ps[:, a:b_],
                            mybir.ActivationFunctionType.Sign,
                            bias=bias36[:], scale=-1.0,
                            accum_out=cnt_sb[:, ccol:ccol + 1],
                        )
                    else:
                        sc = sc_dve_pool.tile([128, WBMAX], f32, tag="sd")
                        nc.vector.tensor_scalar(
                            out=sc[:, a:b_], in0=ps[:, a:b_],
                            scalar1=RCUT2, scalar2=None,
                            op0=mybir.AluOpType.is_lt,
                            op1=mybir.AluOpType.add,
                            accum_out=cnt_sb[:, ccol:ccol + 1],
                        )

            def body(_iv=None):
                for t in range(NQT):
                    do_slot(t)

            if repeat == 1:
                body()
            else:
                with tc.For_i(0, repeat, 1) as iv:
                    body(iv)

            nc.sync.dma_start(out=out_min, in_=min_sb[:])
            nc.sync.dma_start(out=out_cnt, in_=cnt_sb[:])

    nc.compile()
    return nc


def _prep(coord, atype):
    """Host-side prep.

    Returns (in_maps, perms, tile_orders, wslots)."""
    c = np.asarray(coord, dtype=np.float32).reshape(NFRAMES, NLOC, 3)
    at = np.asarray(atype)

    perms, atsorted, csorted = [], [], []
    cand = [[None] * NTILES for _ in range(NFRAMES)]
    for f in range(NFRAMES):
        perm = _kd_perm(c[f])
        perms.append(perm)
        cs = c[f][perm]
        ats = at[f][perm]
        csorted.append(cs)
        atsorted.append(ats)
        cs64 = cs.astype(np.float64)
        for tg in range(NTILES):
            q = cs64[tg * 128:(tg + 1) * 128]
            lo, hi = q.min(0), q.max(0)
            d = np.maximum(np.maximum(lo - cs64, cs64 - hi), 0.0)
            idx = np.nonzero((d * d).sum(1) <= (RCUT + 0.01) ** 2)[0]
            # tile's own atoms first within each type (diagmask window)
            own = (idx >= tg * 128) & (idx < (tg + 1) * 128)
            bytype = []
            for tt in range(NTYPES):
                sel = idx[ats[idx] == tt]
                o = sel[(sel >= tg * 128) & (sel < (tg + 1) * 128)]
                rest = sel[(sel < tg * 128) | (sel >= (tg + 1) * 128)]
                bytype.append(np.concatenate([o, rest]))
            cand[f][tg] = bytype

    # processing order: per core, tiles sorted by type-max width descending
    tile_orders = []       # per core: list of global tile ids in slot order
    for core in range(NCORES):
        f, s = core // CPF, core % CPF
        tiles = list(range(s * NQT, (s + 1) * NQT))
        tiles.sort(key=lambda tg: -max(len(b) for b in cand[f][tg]))
        tile_orders.append(tiles)
    wslots = []
    for t in range(NQT):
        wmax = 0
        for core in range(NCORES):
            f = core // CPF
            tg = tile_orders[core][t]
            wmax = max(wmax, max(len(b) for b in cand[f][tg]))
        wslots.append(max(128, ((wmax + 31) // 32) * 32))
    Ws = [4 * w for w in wslots]
    col0 = np.concatenate([[0], np.cumsum(Ws)]).astype(int)
    WTOT = int(col0[-1])

    ident = np.eye(128, dtype=ml_dtypes.bfloat16)

    in_maps = []
    for core in range(NCORES):
        f, s = core // CPF, core % CPF
        cs64 = csorted[f].astype(np.float64) - CENTER
        qfeat_all, kfeat_all = _features(cs64)
        padf = _features(np.full((1, 3), PAD_COORD - CENTER))[1][:, 0]
        kfeat = np.empty((K, WTOT), ml_dtypes.bfloat16)
        selfpos = np.zeros((128, NQT), np.float32)
        qfeat = np.empty((K, QPC), ml_dtypes.bfloat16)
        for t in range(NQT):
            tg = tile_orders[core][t]
            w = wslots[t]
            qfeat[:, t * 128:(t + 1) * 128] = \
                qfeat_all[:, tg * 128:(tg + 1) * 128]
            for tt in range(NTYPES):
                ids = cand[f][tg][tt]
                n = len(ids)
                seg = int(col0[t]) + tt * w
                kfeat[:, seg:seg + n] = kfeat_all[:, ids]
                kfeat[:, seg + n:seg + w] = padf[:, None]
                # own atoms sit at positions 0..m-1 of this segment
                for j, a_ in enumerate(ids):
                    if tg * 128 <= a_ < (tg + 1) * 128:
                        p = int(a_) - tg * 128
                        selfpos[p, t] = tt * 128 + j
        in_maps.append({
            "qfeat": qfeat,
            "kfeat": kfeat,
            "selfpos": selfpos,
            "ident": ident,
        })
    return in_maps, perms, tile_orders, wslots


def _postprocess(results, perms, tile_orders, wslots, coord, atype):
    c = np.asarray(coord, dtype=np.float32).reshape(NFRAMES, NLOC, 3)
    min_rr2 = np.empty((NFRAMES, NLOC), np.float32)
    max_nnei = np.empty((NFRAMES, NTYPES), np.int64)
    nnei_max = np.zeros((NFRAMES, NTYPES), np.int64)
    mins_sorted = [np.empty(NLOC, np.float32) for _ in range(NFRAMES)]
    for core in range(NCORES):
        f, s = core // CPF, core % CPF
        r = results[core]
        mn = r["out_min"]                      # (128, NQT)
        cnt = r["out_cnt"].reshape(128, NQT, NTYPES)
        for t in range(NQT):
            tg = tile_orders[core][t]
            mins_sorted[f][tg * 128:(tg + 1) * 128] = mn[:, t]
            for tt in range(NTYPES):
                if seg_engine(t, tt) == "A":
                    lt = (cnt[:, t, tt] + wslots[t]) * 0.5
                else:
                    lt = cnt[:, t, tt]
                nnei_max[f, tt] = max(nnei_max[f, tt],
                                      int(np.round(lt.max())))
    for f in range(NFRAMES):
        ms = mins_sorted[f]
        bad = np.nonzero(ms >= RCUT2)[0]
        if len(bad):
            cs = c[f][perms[f]].astype(np.float32)
            for i in bad:
                d = cs - cs[i]
                rr = (d[:, 0] * d[:, 0] + d[:, 1] * d[:, 1]
                      + d[:, 2] * d[:, 2]).astype(np.float32)
                rr[i] = np.inf
                ms[i] = rr.min()
        min_rr2[f, perms[f]] = ms
        max_nnei[f] = nnei_max[f]
    return min_rr2, max_nnei.astype(np.int32)


def kernel(coord, atype):
    from concourse.bass_utils import run_bass_kernel_spmd

    in_maps, perms, tile_orders, wslots = _prep(coord, atype)
    key = tuple(wslots)
    if key not in _CACHE:
        _CACHE[key] = _build(wslots)
    nc = _CACHE[key]
    res = run_bass_kernel_spmd(nc, in_maps, list(range(NCORES)))
    return _postprocess(res.results, perms, tile_orders, wslots,
                        coord, atype)


# revision 16
# speedup vs baseline: 1.7715x; 1.7715x over previous
"""Trainium2 Bass kernel for NeighborStatOP (retrieval_knn).

Computes, for each frame and each local atom i:
  min_rr2[f, i]  = min_{j != i} |x_j - x_i|^2                      (f32)
  max_nnei[f, t] = max_i #{ j != i : |x_j - x_i|^2 < 6^2, type_j = t } (int32)

Strategy (8 NeuronCores, SPMD, one compiled program):
  - Atoms are sorted host-side by a kd-tree (median splits), so each
    128-query tile occupies a compact box.  For every tile the host builds
    a candidate key list: all atoms within RCUT(+eps) of the tile's box
    (exact point-to-box distance).  Only ~400-1300 of the 4096 keys
    survive -> ~3x less on-chip reduction work.  Candidates are grouped by
    atom type into four fixed-width segments per processing slot; each
    core processes its tiles largest-first and slot widths are the
    per-slot maxima over cores, so a single sparse-region outlier tile
    doesn't inflate every slot.  Padding uses far-away dummy atoms.
  - Core c handles frame c//4, query tiles (c%4)*8 .. +8 (kd order).
  - rr2 via tensor-engine matmul: centered per-component expansion
    qc^2 - 2 qc kc + kc^2 with every fp32 feature split into 3 bf16 pieces
    (bf16 x bf16 products are exact in fp32; PSUM accumulates fp32).
    K = 36 rows, full-rate bf16 matmuls.
  - Self-pair knockout: within each type segment the tile's own atoms are
    listed first, so all BIG (1e30) entries live in a 128-wide window at
    each segment start; a small extra matmul per window accumulates the
    per-tile diagmask (BIG at each query's own column) through
    lhsT = identity, start=False.  Position is data, not code.
  - min: DVE tensor_reduce(min) per slot; counts: per type segment a
    ScalarE Sign pass (accum_out; count = (S + W)/2; self/pads give -1 and
    drop out) or a DVE is_lt pass (count = S) - the segment->engine map
    balances ACT and DVE.
  - Host finalizes: count formula, inverse permutations, max; any query
    whose candidate min is >= RCUT^2 (nearest neighbour outside the
    candidate radius; does not occur at realistic densities) is recomputed
    exactly on host.
"""
import sys

sys.path.insert(0, "/opt/trn_rl_repo")

import numpy as np
import ml_dtypes

NFRAMES = 2
NLOC = 4096
NTYPES = 4
RCUT = 6.0
RCUT2 = 36.0
CENTER = 20.0
BIG = 1.0e30
PAD_COORD = 1000.0
NCORES = 8
CPF = 4                       # cores per frame
QPC = NLOC // CPF             # queries per core = 1024
NQT = QPC // 128              # query tiles (slots) per core = 8
NTILES = NLOC // 128          # query tiles per frame = 32
PAIRS = [(0, 0), (0, 1), (1, 0), (1, 1), (0, 2), (2, 0)]
K = 36                        # 12 split rows per component


def seg_engine(t, tau):
    """Engine for the count pass of slot t, type-segment tau (balance)."""
    return "D" if tau == 3 or (tau == 2 and t >= 4) else "A"

_CACHE = {}


def _split3(x):
    p1 = x.astype(ml_dtypes.bfloat16)
    r1 = (x.astype(np.float64) - p1.astype(np.float64)).astype(np.float32)
    p2 = r1.astype(ml_dtypes.bfloat16)
    r2 = (r1.astype(np.float64) - p2.astype(np.float64)).astype(np.float32)
    p3 = r2.astype(ml_dtypes.bfloat16)
    return [p1, p2, p3]


def _features(coords):
    """coords: (n, 3) float64 centered.
    Returns (qfeat (K, n), kfeat (K, n)) as bfloat16 arrays."""
    n = len(coords)
    ones = np.ones(n, ml_dtypes.bfloat16)
    L, R = [], []
    for d in range(3):
        qc = coords[:, d]
        q2 = (qc * qc).astype(np.float32)
        q2p = _split3(q2)
        qp = _split3(qc.astype(np.float32))
        m2p = _split3((-2.0 * qc).astype(np.float32))
        for i in range(3):              # qc^2 pieces x 1
            L.append(q2p[i]); R.append(ones)
        for (ia, ib) in PAIRS:          # qc pieces x -2kc pieces
            L.append(qp[ia]); R.append(m2p[ib])
        for i in range(3):              # 1 x kc^2 pieces
            L.append(ones); R.append(q2p[i])
    return (np.stack(L).astype(ml_dtypes.bfloat16),
            np.stack(R).astype(ml_dtypes.bfloat16))


def _kd_perm(c):
    """c: (n, 3) raw coords. Recursive median split into 128-atom leaves."""
    def split(ids):
        if len(ids) <= 128:
            return [ids]
        spread = c[ids].max(0) - c[ids].min(0)
        ax = int(np.argmax(spread))
        order = ids[np.argsort(c[ids, ax], kind="stable")]
        h = len(ids) // 2
        return split(order[:h]) + split(order[h:])
    return np.concatenate(split(np.arange(len(c))))


def _build(wslots, repeat=1):
    """Build + lower the SPMD kernel. wslots: per-slot segment widths."""
    import concourse.bacc as bacc
    import concourse.tile as tile
    from concourse import mybir

    f32 = mybir.dt.float32
    bf16 = mybir.dt.bfloat16
    Ws = [4 * w for w in wslots]
    col0 = np.concatenate([[0], np.cumsum(Ws)])      # kfeat col offset per slot
    WTOT = int(col0[-1])
    WBMAX = ((max(Ws) + 511) // 512) * 512

    nc = bacc.Bacc("TRN2", target_bir_lowering=False, debug=False,
                   num_devices=NCORES)
    qf = nc.dram_tensor("qfeat", [K, QPC], bf16, kind="ExternalInput").ap()
    kf = nc.dram_tensor("kfeat", [K, WTOT], bf16, kind="ExternalInput").ap()
    sp = nc.dram_tensor("selfpos", [128, NQT], f32,
                        kind="ExternalInput").ap()
    idn = nc.dram_tensor("ident", [128, 128], bf16, kind="ExternalInput").ap()
    out_min = nc.dram_tensor("out_min", [128, NQT], f32,
                             kind="ExternalOutput").ap()
    out_cnt = nc.dram_tensor("out_cnt", [128, NQT * NTYPES], f32,
                             kind="ExternalOutput").ap()

    WBMAX = ((max(4 * w for w in wslots) + 511) // 512) * 512

    # per-slot PSUM tag assignment: dedicated banks per concurrent slot,
    # narrow slots alternate over two buffers; total must fit 8 banks
    WBs = [((4 * w + 511) // 512) * 512 for w in wslots]
    tags, tag_width = [], {}
    budget = 4096
    for t in range(NQT):
        if WBs[t] > 512:
            tag = f"s{t}"
            tags.append(tag)
            tag_width[tag] = WBs[t]
        else:
            tag = f"n{t % 2}"
            tags.append(tag)
            tag_width[tag] = 512
    tot = sum(tag_width.values())
    if tot > 4096:
        # fall back: collapse wide tags pairwise until it fits
        wide_tags = sorted((tg for tg in tag_width if tg.startswith("s")),
                           key=lambda tg: -tag_width[tg])
        i = 1
        while tot > 4096 and i < len(wide_tags):
            victim = wide_tags[i]
            keeper = wide_tags[i - 1]
            for t in range(NQT):
                if tags[t] == victim:
                    tags[t] = keeper
            tot -= tag_width.pop(victim)
            i += 1

    with tile.TileContext(nc) as tc:
        with (
            tc.tile_pool(name="singles", bufs=1) as singles,
            tc.tile_pool(name="psum", bufs=1, space="PSUM") as psum_pool,
            tc.tile_pool(name="dmp", bufs=3) as dm_pool,
            tc.tile_pool(name="sc_act", bufs=3) as sc_act_pool,
            tc.tile_pool(name="sc_dve", bufs=2) as sc_dve_pool,
        ):
            qsb = singles.tile([K, QPC], bf16)
            nc.sync.dma_start(out=qsb[:], in_=qf)
            ksb = singles.tile([K, WTOT], bf16)
            for t in range(NQT):       # per-slot DMAs so slot 0 starts early
                a, b = int(col0[t]), int(col0[t + 1])
                nc.sync.dma_start(out=ksb[:, a:b], in_=kf[:, a:b])
            spsb = singles.tile([128, NQT], f32)
            nc.sync.dma_start(out=spsb[:], in_=sp)
            idsb = singles.tile([128, 128], bf16)
            nc.sync.dma_start(out=idsb[:], in_=idn)
            bias36 = singles.tile([128, 1], f32)
            nc.vector.memset(bias36[:], RCUT2)
            iot = singles.tile([128, NTYPES * 128], mybir.dt.int32)
            nc.gpsimd.iota(iot[:], pattern=[[1, NTYPES * 128]], base=0,
                           channel_multiplier=0)
            min_sb = singles.tile([128, NQT], f32)
            cnt_sb = singles.tile([128, NQT * NTYPES], f32)

            def do_slot(t):
                w = wslots[t]
                W = 4 * w
                k0 = int(col0[t])
                ps = psum_pool.tile([128, tag_width[tags[t]]], f32,
                                    tag=tags[t])
                # diagmask for this slot, built on GPSIMD:
                # dm[p, c] = BIG iff c == selfpos[p, t]
                dmt = dm_pool.tile([128, NTYPES * 128], bf16, tag="dm")
                nc.vector.tensor_scalar(
                    out=dmt[:], in0=iot[:],
                    scalar1=spsb[:, t:t + 1], scalar2=float(BIG),
                    op0=mybir.AluOpType.is_equal,
                    op1=mybir.AluOpType.mult,
                )
                # diag windows: ps range [tau*w, tau*w+min(128,w)) <- dm col
                # tau*128; split at banks, coalesced when contiguous
                pieces = []
                for tau in range(NTYPES):
                    a = tau * w
                    b = a + min(128, w)
                    dc = tau * 128
                    while a < b:
                        e = min(b, ((a // 512) + 1) * 512)
                        pieces.append([a, e, dc])
                        dc += e - a
                        a = e
                merged = [pieces[0]]
                for a, e, dc in pieces[1:]:
                    pa, pe, pdc = merged[-1]
                    if (a == pe and dc == pdc + (pe - pa)
                            and (a % 512) != 0):
                        merged[-1][1] = e
                    else:
                        merged.append([a, e, dc])
                by_bank = {}
                for a, e, dc in merged:
                    by_bank.setdefault(a // 512, []).append((a, e, dc))
                for b0 in range(0, W, 512):
                    b1 = min(b0 + 512, W)
                    nc.tensor.matmul(
                        ps[:, b0:b1],
                        lhsT=qsb[:, t * 128:(t + 1) * 128],
                        rhs=ksb[:, k0 + b0:k0 + b1],
                        start=True, stop=(b0 // 512) not in by_bank,
                    )
                for bank in sorted(by_bank):
                    subs = by_bank[bank]
                    for i, (a, e, dcol) in enumerate(subs):
                        nc.tensor.matmul(
                            ps[:, a:e],
                            lhsT=idsb[:],
                            rhs=dmt[:, dcol:dcol + (e - a)],
                            start=False, stop=(i == len(subs) - 1),
                            skip_group_check=True,
                        )
                nc.vector.tensor_reduce(
                    out=min_sb[:, t:t + 1], in_=ps[:, 0:W],
                    axis=mybir.AxisListType.X, op=mybir.AluOpType.min,
                )
                for s in range(NTYPES):
                    a, b_ = s * w, (s + 1) * w
                    ccol = t * NTYPES + s
                    if seg_engine(t, s) == "A":
                        sc = sc_act_pool.tile([128, WBMAX], f32, tag="sa")
                        nc.scalar.activation(
                            sc[:, a:b_], ps[:, a:b_],
                            mybir.ActivationFunctionType.Sign,
                            bias=bias36[:], scale=-1.0,
                            accum_out=cnt_sb[:, ccol:ccol + 1],
                        )
                    else:
                        sc = sc_dve_pool.tile([128, WBMAX], f32, tag="sd")
                        nc.vector.tensor_scalar(
                            out=sc[:, a:b_], in0=ps[:, a:b_],
                            scalar1=RCUT2, scalar2=None,
                            op0=mybir.AluOpType.is_lt,
                            op1=mybir.AluOpType.add,
                            accum_out=cnt_sb[:, ccol:ccol + 1],
                        )

            def body(_iv=None):
                for t in range(NQT):
                    do_slot(t)

            if repeat == 1:
                body()
            else:
                with tc.For_i(0, repeat, 1) as iv:
                    body(iv)

            nc.sync.dma_start(out=out_min, in_=min_sb[:])
            nc.sync.dma_start(out=out_cnt, in_=cnt_sb[:])

    nc.compile()
    return nc


def _prep(coord, atype):
    """Host-side prep.

    Returns (in_maps, perms, tile_orders, wslots)."""
    c = np.asarray(coord, dtype=np.float32).reshape(NFRAMES, NLOC, 3)
    at = np.asarray(atype)

    perms, atsorted, csorted = [], [], []
    cand = [[None] * NTILES for _ in range(NFRAMES)]
    for f in range(NFRAMES):
        perm = _kd_perm(c[f])
        perms.append(perm)
        cs = c[f][perm]
        ats = at[f][perm]
        csorted.append(cs)
        atsorted.append(ats)
        cs64 = cs.astype(np.float64)
        for tg in range(NTILES):
            q = cs64[tg * 128:(tg + 1) * 128]
            lo, hi = q.min(0), q.max(0)
            d = np.maximum(np.maximum(lo - cs64, cs64 - hi), 0.0)
            idx = np.nonzero((d * d).sum(1) <= (RCUT + 0.01) ** 2)[0]
            # tile's own atoms first within each type (diagmask window)
            own = (idx >= tg * 128) & (idx < (tg + 1) * 128)
            bytype = []
            for tt in range(NTYPES):
                sel = idx[ats[idx] == tt]
                o = sel[(sel >= tg * 128) & (sel < (tg + 1) * 128)]
                rest = sel[(sel < tg * 128) | (sel >= (tg + 1) * 128)]
                bytype.append(np.concatenate([o, rest]))
            cand[f][tg] = bytype

    # processing order: per core, tiles sorted by type-max width descending
    tile_orders = []       # per core: list of global tile ids in slot order
    for core in range(NCORES):
        f, s = core // CPF, core % CPF
        tiles = list(range(s * NQT, (s + 1) * NQT))
        tiles.sort(key=lambda tg: -max(len(b) for b in cand[f][tg]))
        tile_orders.append(tiles)
    wslots = []
    for t in range(NQT):
        wmax = 0
        for core in range(NCORES):
            f = core // CPF
            tg = tile_orders[core][t]
            wmax = max(wmax, max(len(b) for b in cand[f][tg]))
        wslots.append(max(128, ((wmax + 31) // 32) * 32))
    Ws = [4 * w for w in wslots]
    col0 = np.concatenate([[0], np.cumsum(Ws)]).astype(int)
    WTOT = int(col0[-1])

    ident = np.eye(128, dtype=ml_dtypes.bfloat16)

    in_maps = []
    for core in range(NCORES):
        f, s = core // CPF, core % CPF
        cs64 = csorted[f].astype(np.float64) - CENTER
        qfeat_all, kfeat_all = _features(cs64)
        padf = _features(np.full((1, 3), PAD_COORD - CENTER))[1][:, 0]
        kfeat = np.empty((K, WTOT), ml_dtypes.bfloat16)
        selfpos = np.zeros((128, NQT), np.float32)
        qfeat = np.empty((K, QPC), ml_dtypes.bfloat16)
        for t in range(NQT):
            tg = tile_orders[core][t]
            w = wslots[t]
            qfeat[:, t * 128:(t + 1) * 128] = \
                qfeat_all[:, tg * 128:(tg + 1) * 128]
            for tt in range(NTYPES):
                ids = cand[f][tg][tt]
                n = len(ids)
                seg = int(col0[t]) + tt * w
                kfeat[:, seg:seg + n] = kfeat_all[:, ids]
                kfeat[:, seg + n:seg + w] = padf[:, None]
                # own atoms sit at positions 0..m-1 of this segment
                for j, a_ in enumerate(ids):
                    if tg * 128 <= a_ < (tg + 1) * 128:
                        p = int(a_) - tg * 128
                        selfpos[p, t] = tt * 128 + j
        in_maps.append({
            "qfeat": qfeat,
            "kfeat": kfeat,
            "selfpos": selfpos,
            "ident": ident,
        })
    return in_maps, perms, tile_orders, wslots


def _postprocess(results, perms, tile_orders, wslots, coord, atype):
    c = np.asarray(coord, dtype=np.float32).reshape(NFRAMES, NLOC, 3)
    min_rr2 = np.empty((NFRAMES, NLOC), np.float32)
    max_nnei = np.empty((NFRAMES, NTYPES), np.int64)
    nnei_max = np.zeros((NFRAMES, NTYPES), np.int64)
    mins_sorted = [np.empty(NLOC, np.float32) for _ in range(NFRAMES)]
    for core in range(NCORES):
        f, s = core // CPF, core % CPF
        r = results[core]
        mn = r["out_min"]                      # (128, NQT)
        cnt = r["out_cnt"].reshape(128, NQT, NTYPES)
        for t in range(NQT):
            tg = tile_orders[core][t]
            mins_sorted[f][tg * 128:(tg + 1) * 128] = mn[:, t]
            for tt in range(NTYPES):
                if seg_engine(t, tt) == "A":
                    lt = (cnt[:, t, tt] + wslots[t]) * 0.5
                else:
                    lt = cnt[:, t, tt]
                nnei_max[f, tt] = max(nnei_max[f, tt],
                                      int(np.round(lt.max())))
    for f in range(NFRAMES):
        ms = mins_sorted[f]
        bad = np.nonzero(ms >= RCUT2)[0]
        if len(bad):
            cs = c[f][perms[f]].astype(np.float32)
            for i in bad:
                d = cs - cs[i]
                rr = (d[:, 0] * d[:, 0] + d[:, 1] * d[:, 1]
                      + d[:, 2] * d[:, 2]).astype(np.float32)
                rr[i] = np.inf
                ms[i] = rr.min()
        min_rr2[f, perms[f]] = ms
        max_nnei[f] = nnei_max[f]
    return min_rr2, max_nnei.astype(np.int32)


def kernel(coord, atype):
    from concourse.bass_utils import run_bass_kernel_spmd

    in_maps, perms, tile_orders, wslots = _prep(coord, atype)
    key = tuple(wslots)
    if key not in _CACHE:
        _CACHE[key] = _build(wslots)
    nc = _CACHE[key]
    res = run_bass_kernel_spmd(nc, in_maps, list(range(NCORES)))
    return _postprocess(res.results, perms, tile_orders, wslots,
                        coord, atype)


# revision 20
# speedup vs baseline: 2.5222x; 1.4237x over previous
"""Trainium2 Bass kernel for NeighborStatOP (retrieval_knn).

Computes, for each frame and each local atom i:
  min_rr2[f, i]  = min_{j != i} |x_j - x_i|^2                      (f32)
  max_nnei[f, t] = max_i #{ j != i : |x_j - x_i|^2 < 6^2, type_j = t } (int32)

Strategy (8 NeuronCores, SPMD, one compiled program):
  - Atoms are sorted host-side by a kd-tree (median splits), so each
    128-query tile occupies a compact box.  For every tile the host builds
    a candidate key list: all atoms within RCUT(+eps) of the tile's box
    (exact point-to-box distance).  Only ~400-1300 of the 4096 keys
    survive -> ~3x less on-chip reduction work.  Candidates are grouped by
    atom type into four fixed-width segments per processing slot; each
    core processes its tiles largest-first and slot widths are the
    per-slot maxima over cores, so a single sparse-region outlier tile
    doesn't inflate every slot.  Padding uses far-away dummy atoms.
  - Core c handles frame c//4, query tiles (c%4)*8 .. +8 (kd order).
  - rr2 via tensor-engine matmul: centered per-component expansion
    qc^2 - 2 qc kc + kc^2 with every fp32 feature split into 3 bf16 pieces
    (bf16 x bf16 products are exact in fp32; PSUM accumulates fp32).
    K = 36 rows, full-rate bf16 matmuls.
  - Self-pair knockout: within each type segment the tile's own atoms are
    listed first, so all BIG (1e30) entries live in a 128-wide window at
    each segment start; a small extra matmul per window accumulates the
    per-tile diagmask (BIG at each query's own column) through
    lhsT = identity, start=False.  Position is data, not code.
  - min: DVE tensor_reduce(min) per slot; counts: per type segment a
    ScalarE Sign pass (accum_out; count = (S + W)/2; self/pads give -1 and
    drop out) or a DVE is_lt pass (count = S) - the segment->engine map
    balances ACT and DVE.
  - Host finalizes: count formula, inverse permutations, max; any query
    whose candidate min is >= RCUT^2 (nearest neighbour outside the
    candidate radius; does not occur at realistic densities) is recomputed
    exactly on host.
"""
import sys

sys.path.insert(0, "/opt/trn_rl_repo")

import numpy as np
import ml_dtypes

NFRAMES = 2
NLOC = 4096
NTYPES = 4
RCUT = 6.0
RCUT2 = 36.0
CENTER = 20.0
BIG = 1.0e30
PAD_COORD = 1000.0
NCORES = 8
CPF = 4                       # cores per frame
QPC = NLOC // CPF             # queries per core = 1024
NQT = QPC // 128              # query tiles (slots) per core = 8
NTILES = NLOC // 128          # query tiles per frame = 32
PAIRS = [(0, 0), (0, 1), (1, 0), (1, 1), (0, 2), (2, 0)]
K = 36                        # 12 split rows per component


def seg_engine(t, tau):
    """Engine for the count pass of slot t, type-segment tau (balance)."""
    return "D" if tau == 3 or (tau == 2 and t >= 4) else "A"

_CACHE = {}


def _split3(x):
    p1 = x.astype(ml_dtypes.bfloat16)
    r1 = (x.astype(np.float64) - p1.astype(np.float64)).astype(np.float32)
    p2 = r1.astype(ml_dtypes.bfloat16)
    r2 = (r1.astype(np.float64) - p2.astype(np.float64)).astype(np.float32)
    p3 = r2.astype(ml_dtypes.bfloat16)
    return [p1, p2, p3]


def _features(coords):
    """coords: (n, 3) float64 centered.
    Returns (qfeat (K, n), kfeat (K, n)) as bfloat16 arrays."""
    n = len(coords)
    ones = np.ones(n, ml_dtypes.bfloat16)
    L, R = [], []
    for d in range(3):
        qc = coords[:, d]
        q2 = (qc * qc).astype(np.float32)
        q2p = _split3(q2)
        qp = _split3(qc.astype(np.float32))
        m2p = _split3((-2.0 * qc).astype(np.float32))
        for i in range(3):              # qc^2 pieces x 1
            L.append(q2p[i]); R.append(ones)
        for (ia, ib) in PAIRS:          # qc pieces x -2kc pieces
            L.append(qp[ia]); R.append(m2p[ib])
        for i in range(3):              # 1 x kc^2 pieces
            L.append(ones); R.append(q2p[i])
    return (np.stack(L).astype(ml_dtypes.bfloat16),
            np.stack(R).astype(ml_dtypes.bfloat16))


def _kd_perm(c):
    """c: (n, 3) raw coords. Recursive median split into 128-atom leaves."""
    def split(ids):
        if len(ids) <= 128:
            return [ids]
        spread = c[ids].max(0) - c[ids].min(0)
        ax = int(np.argmax(spread))
        order = ids[np.argsort(c[ids, ax], kind="stable")]
        h = len(ids) // 2
        return split(order[:h]) + split(order[h:])
    return np.concatenate(split(np.arange(len(c))))


def _build(wslots, repeat=1):
    """Build + lower the SPMD kernel. wslots: per-slot segment widths."""
    import concourse.bacc as bacc
    import concourse.tile as tile
    from concourse import mybir

    f32 = mybir.dt.float32
    bf16 = mybir.dt.bfloat16
    Ws = [4 * w for w in wslots]
    col0 = np.concatenate([[0], np.cumsum(Ws)])      # kfeat col offset per slot
    WTOT = int(col0[-1])
    WBMAX = ((max(Ws) + 511) // 512) * 512

    nc = bacc.Bacc("TRN2", target_bir_lowering=False, debug=False,
                   num_devices=NCORES)
    qf = nc.dram_tensor("qfeat", [K, QPC], bf16, kind="ExternalInput").ap()
    kf = nc.dram_tensor("kfeat", [K, WTOT], bf16, kind="ExternalInput").ap()
    sp = nc.dram_tensor("selfpos", [128, NQT], f32,
                        kind="ExternalInput").ap()
    idn = nc.dram_tensor("ident", [128, 128], bf16, kind="ExternalInput").ap()
    out_min = nc.dram_tensor("out_min", [128, NQT], f32,
                             kind="ExternalOutput").ap()
    out_cnt = nc.dram_tensor("out_cnt", [128, NQT * NTYPES], f32,
                             kind="ExternalOutput").ap()

    WBMAX = ((max(4 * w for w in wslots) + 511) // 512) * 512

    # per-slot PSUM tag assignment: dedicated banks per concurrent slot,
    # narrow slots alternate over two buffers; total must fit 8 banks
    WBs = [((4 * w + 511) // 512) * 512 for w in wslots]
    tags, tag_width = [], {}
    budget = 4096
    for t in range(NQT):
        if WBs[t] > 512:
            tag = f"s{t}"
            tags.append(tag)
            tag_width[tag] = WBs[t]
        else:
            tag = f"n{t % 2}"
            tags.append(tag)
            tag_width[tag] = 512
    tot = sum(tag_width.values())
    if tot > 4096:
        # fall back: collapse wide tags pairwise until it fits,
        # starting from the narrowest (they serialize the least work)
        wide_tags = sorted((tg for tg in tag_width if tg.startswith("s")),
                           key=lambda tg: tag_width[tg])
        i = 1
        while tot > 4096 and i < len(wide_tags):
            victim = wide_tags[i]
            keeper = wide_tags[i - 1]
            for t in range(NQT):
                if tags[t] == victim:
                    tags[t] = keeper
            tot -= tag_width.pop(victim)
            i += 1

    with tile.TileContext(nc) as tc:
        with (
            tc.tile_pool(name="singles", bufs=1) as singles,
            tc.tile_pool(name="psum", bufs=1, space="PSUM") as psum_pool,
            tc.tile_pool(name="dmp", bufs=2) as dm_pool,
            tc.tile_pool(name="sc_act", bufs=3) as sc_act_pool,
            tc.tile_pool(name="sc_dve", bufs=2) as sc_dve_pool,
        ):
            qsb = singles.tile([K, QPC], bf16)
            nc.sync.dma_start(out=qsb[:], in_=qf)
            ksb = singles.tile([K, WTOT], bf16)
            for t in range(NQT):       # per-slot DMAs so slot 0 starts early
                a, b = int(col0[t]), int(col0[t + 1])
                nc.sync.dma_start(out=ksb[:, a:b], in_=kf[:, a:b])
            spsb = singles.tile([128, NQT], f32)
            nc.sync.dma_start(out=spsb[:], in_=sp)
            idsb = singles.tile([128, 128], bf16)
            nc.sync.dma_start(out=idsb[:], in_=idn)
            bias36 = singles.tile([128, 1], f32)
            nc.vector.memset(bias36[:], RCUT2)
            warm = singles.tile([128, 1], f32)
            nc.scalar.activation(warm[:], bias36[:],
                                 mybir.ActivationFunctionType.Sign,
                                 bias=bias36[:], scale=-1.0)
            iot = singles.tile([128, NTYPES * 128], mybir.dt.int32)
            nc.gpsimd.iota(iot[:], pattern=[[1, NTYPES * 128]], base=0,
                           channel_multiplier=0)
            min_sb = singles.tile([128, NQT], f32)
            cnt_sb = singles.tile([128, NQT * NTYPES], f32)

            def make_dm(t):
                # diagmask for slot t: dm[p, c] = BIG iff c == selfpos[p, t]
                dmt = dm_pool.tile([128, NTYPES * 128], bf16, tag="dm")
                nc.vector.tensor_scalar(
                    out=dmt[:], in0=iot[:],
                    scalar1=spsb[:, t:t + 1], scalar2=float(BIG),
                    op0=mybir.AluOpType.is_equal,
                    op1=mybir.AluOpType.mult,
                )
                return dmt

            def do_slot(t, dmt):
                w = wslots[t]
                W = 4 * w
                k0 = int(col0[t])
                ps = psum_pool.tile([128, tag_width[tags[t]]], f32,
                                    tag=tags[t])
                # diag windows: ps range [tau*w, tau*w+min(128,w)) <- dm col
                # tau*128; split at banks, coalesced when contiguous
                pieces = []
                for tau in range(NTYPES):
                    a = tau * w
                    b = a + min(128, w)
                    dc = tau * 128
                    while a < b:
                        e = min(b, ((a // 512) + 1) * 512)
                        pieces.append([a, e, dc])
                        dc += e - a
                        a = e
                merged = [pieces[0]]
                for a, e, dc in pieces[1:]:
                    pa, pe, pdc = merged[-1]
                    if (a == pe and dc == pdc + (pe - pa)
                            and (a % 512) != 0):
                        merged[-1][1] = e
                    else:
                        merged.append([a, e, dc])
                by_bank = {}
                for a, e, dc in merged:
                    by_bank.setdefault(a // 512, []).append((a, e, dc))
                for b0 in range(0, W, 512):
                    b1 = min(b0 + 512, W)
                    nc.tensor.matmul(
                        ps[:, b0:b1],
                        lhsT=qsb[:, t * 128:(t + 1) * 128],
                        rhs=ksb[:, k0 + b0:k0 + b1],
                        start=True, stop=(b0 // 512) not in by_bank,
                    )
                for bank in sorted(by_bank):
                    subs = by_bank[bank]
                    for i, (a, e, dcol) in enumerate(subs):
                        nc.tensor.matmul(
                            ps[:, a:e],
                            lhsT=idsb[:],
                            rhs=dmt[:, dcol:dcol + (e - a)],
                            start=False, stop=(i == len(subs) - 1),
                            skip_group_check=True,
                        )
                nc.vector.tensor_reduce(
                    out=min_sb[:, t:t + 1], in_=ps[:, 0:W],
                    axis=mybir.AxisListType.X, op=mybir.AluOpType.min,
                )
                for s in range(NTYPES):
                    a, b_ = s * w, (s + 1) * w
                    ccol = t * NTYPES + s
                    if seg_engine(t, s) == "A":
                        sc = sc_act_pool.tile([128, WBMAX], f32, tag="sa")
                        nc.scalar.activation(
                            sc[:, a:b_], ps[:, a:b_],
                            mybir.ActivationFunctionType.Sign,
                            bias=bias36[:], scale=-1.0,
                            accum_out=cnt_sb[:, ccol:ccol + 1],
                        )
                    else:
                        sc = sc_dve_pool.tile([128, WBMAX], f32, tag="sd")
                        nc.vector.tensor_scalar(
                            out=sc[:, a:b_], in0=ps[:, a:b_],
                            scalar1=RCUT2, scalar2=None,
                            op0=mybir.AluOpType.is_lt,
                            op1=mybir.AluOpType.add,
                            accum_out=cnt_sb[:, ccol:ccol + 1],
                        )
                nc.sync.dma_start(out=out_min[:, t:t + 1],
                                  in_=min_sb[:, t:t + 1])
                nc.sync.dma_start(
                    out=out_cnt[:, t * NTYPES:(t + 1) * NTYPES],
                    in_=cnt_sb[:, t * NTYPES:(t + 1) * NTYPES])

            def body(_iv=None):
                for t in range(NQT):
                    do_slot(t, make_dm(t))

            if repeat == 1:
                body()
            else:
                with tc.For_i(0, repeat, 1) as iv:
                    body(iv)



    nc.compile()
    return nc


def _prep(coord, atype):
    """Host-side prep.

    Returns (in_maps, perms, tile_orders, wslots)."""
    c = np.asarray(coord, dtype=np.float32).reshape(NFRAMES, NLOC, 3)
    at = np.asarray(atype)

    perms, atsorted, csorted = [], [], []
    cand = [[None] * NTILES for _ in range(NFRAMES)]
    for f in range(NFRAMES):
        perm = _kd_perm(c[f])
        perms.append(perm)
        cs = c[f][perm]
        ats = at[f][perm]
        csorted.append(cs)
        atsorted.append(ats)
        cs64 = cs.astype(np.float64)
        for tg in range(NTILES):
            q = cs64[tg * 128:(tg + 1) * 128]
            lo, hi = q.min(0), q.max(0)
            d = np.maximum(np.maximum(lo - cs64, cs64 - hi), 0.0)
            idx = np.nonzero((d * d).sum(1) <= (RCUT + 0.01) ** 2)[0]
            # tile's own atoms first within each type (diagmask window)
            own = (idx >= tg * 128) & (idx < (tg + 1) * 128)
            bytype = []
            for tt in range(NTYPES):
                sel = idx[ats[idx] == tt]
                o = sel[(sel >= tg * 128) & (sel < (tg + 1) * 128)]
                rest = sel[(sel < tg * 128) | (sel >= (tg + 1) * 128)]
                bytype.append(np.concatenate([o, rest]))
            cand[f][tg] = bytype

    # processing order: per core, tiles sorted by type-max width descending
    tile_orders = []       # per core: list of global tile ids in slot order
    for core in range(NCORES):
        f, s = core // CPF, core % CPF
        tiles = list(range(s * NQT, (s + 1) * NQT))
        tiles.sort(key=lambda tg: -max(len(b) for b in cand[f][tg]))
        tile_orders.append(tiles)
    wslots = []
    for t in range(NQT):
        wmax = 0
        for core in range(NCORES):
            f = core // CPF
            tg = tile_orders[core][t]
            wmax = max(wmax, max(len(b) for b in cand[f][tg]))
        wslots.append(max(128, ((wmax + 31) // 32) * 32))
    Ws = [4 * w for w in wslots]
    col0 = np.concatenate([[0], np.cumsum(Ws)]).astype(int)
    WTOT = int(col0[-1])

    ident = np.eye(128, dtype=ml_dtypes.bfloat16)

    in_maps = []
    for core in range(NCORES):
        f, s = core // CPF, core % CPF
        cs64 = csorted[f].astype(np.float64) - CENTER
        qfeat_all, kfeat_all = _features(cs64)
        padf = _features(np.full((1, 3), PAD_COORD - CENTER))[1][:, 0]
        kfeat = np.empty((K, WTOT), ml_dtypes.bfloat16)
        selfpos = np.zeros((128, NQT), np.float32)
        qfeat = np.empty((K, QPC), ml_dtypes.bfloat16)
        for t in range(NQT):
            tg = tile_orders[core][t]
            w = wslots[t]
            qfeat[:, t * 128:(t + 1) * 128] = \
                qfeat_all[:, tg * 128:(tg + 1) * 128]
            for tt in range(NTYPES):
                ids = cand[f][tg][tt]
                n = len(ids)
                seg = int(col0[t]) + tt * w
                kfeat[:, seg:seg + n] = kfeat_all[:, ids]
                kfeat[:, seg + n:seg + w] = padf[:, None]
                # own atoms sit at positions 0..m-1 of this segment
                for j, a_ in enumerate(ids):
                    if tg * 128 <= a_ < (tg + 1) * 128:
                        p = int(a_) - tg * 128
                        selfpos[p, t] = tt * 128 + j
        in_maps.append({
            "qfeat": qfeat,
            "kfeat": kfeat,
            "selfpos": selfpos,
            "ident": ident,
        })
    return in_maps, perms, tile_orders, wslots


def _postprocess(results, perms, tile_orders, wslots, coord, atype):
    c = np.asarray(coord, dtype=np.float32).reshape(NFRAMES, NLOC, 3)
    min_rr2 = np.empty((NFRAMES, NLOC), np.float32)
    max_nnei = np.empty((NFRAMES, NTYPES), np.int64)
    nnei_max = np.zeros((NFRAMES, NTYPES), np.int64)
    mins_sorted = [np.empty(NLOC, np.float32) for _ in range(NFRAMES)]
    for core in range(NCORES):
        f, s = core // CPF, core % CPF
        r = results[core]
        mn = r["out_min"]                      # (128, NQT)
        cnt = r["out_cnt"].reshape(128, NQT, NTYPES)
        for t in range(NQT):
            tg = tile_orders[core][t]
            mins_sorted[f][tg * 128:(tg + 1) * 128] = mn[:, t]
            for tt in range(NTYPES):
                if seg_engine(t, tt) == "A":
                    lt = (cnt[:, t, tt] + wslots[t]) * 0.5
                else:
                    lt = cnt[:, t, tt]
                nnei_max[f, tt] = max(nnei_max[f, tt],
                                      int(np.round(lt.max())))
    for f in range(NFRAMES):
        ms = mins_sorted[f]
        bad = np.nonzero(ms >= RCUT2)[0]
        if len(bad):
            cs = c[f][perms[f]].astype(np.float32)
            for i in bad:
                d = cs - cs[i]
                rr = (d[:, 0] * d[:, 0] + d[:, 1] * d[:, 1]
                      + d[:, 2] * d[:, 2]).astype(np.float32)
                rr[i] = np.inf
                ms[i] = rr.min()
        min_rr2[f, perms[f]] = ms
        max_nnei[f] = nnei_max[f]
    return min_rr2, max_nnei.astype(np.int32)


def kernel(coord, atype):
    from concourse.bass_utils import run_bass_kernel_spmd

    in_maps, perms, tile_orders, wslots = _prep(coord, atype)
    key = tuple(wslots)
    if key not in _CACHE:
        _CACHE[key] = _build(wslots)
    nc = _CACHE[key]
    res = run_bass_kernel_spmd(nc, in_maps, list(range(NCORES)))
    return _postprocess(res.results, perms, tile_orders, wslots,
                        coord, atype)


# revision 21
# speedup vs baseline: 4.0364x; 1.6003x over previous
"""Trainium2 Bass kernel for NeighborStatOP (retrieval_knn).

Computes, for each frame and each local atom i:
  min_rr2[f, i]  = min_{j != i} |x_j - x_i|^2                      (f32)
  max_nnei[f, t] = max_i #{ j != i : |x_j - x_i|^2 < 6^2, type_j = t } (int32)

Strategy (8 NeuronCores, SPMD, one compiled program):
  - Atoms are sorted host-side by a kd-tree (median splits), so each
    128-query tile occupies a compact box.  For every tile the host builds
    a candidate key list: all atoms within RCUT(+eps) of the tile's box
    (exact point-to-box distance).  Only ~400-1300 of the 4096 keys
    survive -> ~3x less on-chip reduction work.  Candidates are grouped by
    atom type into four fixed-width segments per processing slot; each
    core processes its tiles largest-first and slot widths are the
    per-slot maxima over cores, so a single sparse-region outlier tile
    doesn't inflate every slot.  Padding uses far-away dummy atoms.
  - Core c handles frame c//4, query tiles (c%4)*8 .. +8 (kd order).
  - rr2 via tensor-engine matmul: centered per-component expansion
    qc^2 - 2 qc kc + kc^2 with every fp32 feature split into 3 bf16 pieces
    (bf16 x bf16 products are exact in fp32; PSUM accumulates fp32).
    K = 36 rows, full-rate bf16 matmuls.
  - Self-pair knockout: within each type segment the tile's own atoms are
    listed first, so all BIG (1e30) entries live in a 128-wide window at
    each segment start; a small extra matmul per window accumulates the
    per-tile diagmask (BIG at each query's own column) through
    lhsT = identity, start=False.  Position is data, not code.
  - min: DVE tensor_reduce(min) per slot; counts: per type segment a
    ScalarE Sign pass (accum_out; count = (S + W)/2; self/pads give -1 and
    drop out) or a DVE is_lt pass (count = S) - the segment->engine map
    balances ACT and DVE.
  - Host finalizes: count formula, inverse permutations, max; any query
    whose candidate min is >= RCUT^2 (nearest neighbour outside the
    candidate radius; does not occur at realistic densities) is recomputed
    exactly on host.
"""
import sys

sys.path.insert(0, "/opt/trn_rl_repo")

import numpy as np
import ml_dtypes

NFRAMES = 2
NLOC = 4096
NTYPES = 4
RCUT = 6.0
RCUT2 = 36.0
CENTER = 20.0
BIG = 1.0e30
PAD_COORD = 1000.0
NCORES = 8
CPF = 4                       # cores per frame
QPC = NLOC // CPF             # queries per core = 1024
NQT = QPC // 128              # query tiles (slots) per core = 8
NTILES = NLOC // 128          # query tiles per frame = 32
PAIRS = [(0, 0), (0, 1), (1, 0), (1, 1), (0, 2), (2, 0)]
K = 36                        # 12 split rows per component


def seg_engine(t, tau):
    """Engine for the count pass of slot t, type-segment tau (balance)."""
    return "D" if tau == 3 or (tau == 2 and t >= 4) else "A"

_CACHE = {}


def _split3(x):
    p1 = x.astype(ml_dtypes.bfloat16)
    r1 = (x.astype(np.float64) - p1.astype(np.float64)).astype(np.float32)
    p2 = r1.astype(ml_dtypes.bfloat16)
    r2 = (r1.astype(np.float64) - p2.astype(np.float64)).astype(np.float32)
    p3 = r2.astype(ml_dtypes.bfloat16)
    return [p1, p2, p3]


def _features(coords):
    """coords: (n, 3) float64 centered.
    Returns (qfeat (K, n), kfeat (K, n)) as bfloat16 arrays."""
    n = len(coords)
    ones = np.ones(n, ml_dtypes.bfloat16)
    L, R = [], []
    for d in range(3):
        qc = coords[:, d]
        q2 = (qc * qc).astype(np.float32)
        q2p = _split3(q2)
        qp = _split3(qc.astype(np.float32))
        m2p = _split3((-2.0 * qc).astype(np.float32))
        for i in range(3):              # qc^2 pieces x 1
            L.append(q2p[i]); R.append(ones)
        for (ia, ib) in PAIRS:          # qc pieces x -2kc pieces
            L.append(qp[ia]); R.append(m2p[ib])
        for i in range(3):              # 1 x kc^2 pieces
            L.append(ones); R.append(q2p[i])
    return (np.stack(L).astype(ml_dtypes.bfloat16),
            np.stack(R).astype(ml_dtypes.bfloat16))


def _kd_perm(c):
    """c: (n, 3) raw coords. Recursive median split into 128-atom leaves."""
    def split(ids):
        if len(ids) <= 128:
            return [ids]
        spread = c[ids].max(0) - c[ids].min(0)
        ax = int(np.argmax(spread))
        order = ids[np.argsort(c[ids, ax], kind="stable")]
        h = len(ids) // 2
        return split(order[:h]) + split(order[h:])
    return np.concatenate(split(np.arange(len(c))))


def _build(wslots, repeat=1):
    """Build + lower the SPMD kernel. wslots: per-slot segment widths."""
    import concourse.bacc as bacc
    import concourse.tile as tile
    from concourse import mybir

    f32 = mybir.dt.float32
    bf16 = mybir.dt.bfloat16
    Ws = [4 * w for w in wslots]
    col0 = np.concatenate([[0], np.cumsum(Ws)])      # kfeat col offset per slot
    WTOT = int(col0[-1])
    WBMAX = ((max(Ws) + 511) // 512) * 512

    nc = bacc.Bacc("TRN2", target_bir_lowering=False, debug=False,
                   num_devices=NCORES)
    qf = nc.dram_tensor("qfeat", [K, QPC], bf16, kind="ExternalInput").ap()
    kf = nc.dram_tensor("kfeat", [K, WTOT], bf16, kind="ExternalInput").ap()
    sp = nc.dram_tensor("selfpos", [128, NQT], f32,
                        kind="ExternalInput").ap()
    idn = nc.dram_tensor("ident", [128, 128], bf16, kind="ExternalInput").ap()
    out_min = nc.dram_tensor("out_min", [128, NQT], f32,
                             kind="ExternalOutput").ap()
    out_cnt = nc.dram_tensor("out_cnt", [128, NQT * NTYPES], f32,
                             kind="ExternalOutput").ap()

    WBMAX = ((max(4 * w for w in wslots) + 511) // 512) * 512

    # per-slot PSUM tag assignment: dedicated banks per concurrent slot,
    # narrow slots alternate over two buffers; total must fit 8 banks
    WBs = [((4 * w + 511) // 512) * 512 for w in wslots]
    tags, tag_width = [], {}
    budget = 4096
    for t in range(NQT):
        if WBs[t] > 512:
            tag = f"s{t}"
            tags.append(tag)
            tag_width[tag] = WBs[t]
        else:
            tag = f"n{t % 3}"
            tags.append(tag)
            tag_width[tag] = 512
    tot = sum(tag_width.values())
    if tot > 4096:
        # fall back: collapse wide tags pairwise until it fits,
        # starting from the narrowest (they serialize the least work)
        wide_tags = sorted((tg for tg in tag_width if tg.startswith("s")),
                           key=lambda tg: tag_width[tg])
        i = 1
        while tot > 4096 and i < len(wide_tags):
            victim = wide_tags[i]
            keeper = wide_tags[i - 1]
            for t in range(NQT):
                if tags[t] == victim:
                    tags[t] = keeper
            tot -= tag_width.pop(victim)
            i += 1

    with tile.TileContext(nc) as tc:
        with (
            tc.tile_pool(name="singles", bufs=1) as singles,
            tc.tile_pool(name="psum", bufs=1, space="PSUM") as psum_pool,
            tc.tile_pool(name="dmp", bufs=2) as dm_pool,
            tc.tile_pool(name="sc_act", bufs=3) as sc_act_pool,
            tc.tile_pool(name="sc_dve", bufs=2) as sc_dve_pool,
        ):
            qsb = singles.tile([K, QPC], bf16)
            nc.sync.dma_start(out=qsb[:], in_=qf)
            ksb = singles.tile([K, WTOT], bf16)
            for t in range(NQT):       # per-slot DMAs so slot 0 starts early
                a, b = int(col0[t]), int(col0[t + 1])
                nc.sync.dma_start(out=ksb[:, a:b], in_=kf[:, a:b])
            spsb = singles.tile([128, NQT], f32)
            nc.sync.dma_start(out=spsb[:], in_=sp)
            idsb = singles.tile([128, 128], bf16)
            nc.sync.dma_start(out=idsb[:], in_=idn)
            bias36 = singles.tile([128, 1], f32)
            nc.vector.memset(bias36[:], RCUT2)
            warm = singles.tile([128, 1], f32)
            nc.scalar.activation(warm[:], bias36[:],
                                 mybir.ActivationFunctionType.Sign,
                                 bias=bias36[:], scale=-1.0)
            iot = singles.tile([128, NTYPES * 128], mybir.dt.int32)
            nc.gpsimd.iota(iot[:], pattern=[[1, NTYPES * 128]], base=0,
                           channel_multiplier=0)
            min_sb = singles.tile([128, NQT], f32)
            cnt_sb = singles.tile([128, NQT * NTYPES], f32)

            def make_dm(t):
                # diagmask for slot t: dm[p, c] = BIG iff c == selfpos[p, t]
                dmt = dm_pool.tile([128, NTYPES * 128], bf16, tag="dm")
                nc.vector.tensor_scalar(
                    out=dmt[:], in0=iot[:],
                    scalar1=spsb[:, t:t + 1], scalar2=float(BIG),
                    op0=mybir.AluOpType.is_equal,
                    op1=mybir.AluOpType.mult,
                )
                return dmt

            def do_slot(t, dmt):
                w = wslots[t]
                W = 4 * w
                k0 = int(col0[t])
                ps = psum_pool.tile([128, tag_width[tags[t]]], f32,
                                    tag=tags[t])
                # diag windows: ps range [tau*w, tau*w+min(128,w)) <- dm col
                # tau*128; split at banks, coalesced when contiguous
                pieces = []
                for tau in range(NTYPES):
                    a = tau * w
                    b = a + min(128, w)
                    dc = tau * 128
                    while a < b:
                        e = min(b, ((a // 512) + 1) * 512)
                        pieces.append([a, e, dc])
                        dc += e - a
                        a = e
                merged = [pieces[0]]
                for a, e, dc in pieces[1:]:
                    pa, pe, pdc = merged[-1]
                    if (a == pe and dc == pdc + (pe - pa)
                            and (a % 512) != 0):
                        merged[-1][1] = e
                    else:
                        merged.append([a, e, dc])
                by_bank = {}
                for a, e, dc in merged:
                    by_bank.setdefault(a // 512, []).append((a, e, dc))
                for b0 in range(0, W, 512):
                    b1 = min(b0 + 512, W)
                    nc.tensor.matmul(
                        ps[:, b0:b1],
                        lhsT=qsb[:, t * 128:(t + 1) * 128],
                        rhs=ksb[:, k0 + b0:k0 + b1],
                        start=True, stop=(b0 // 512) not in by_bank,
                    )
                for bank in sorted(by_bank):
                    subs = by_bank[bank]
                    for i, (a, e, dcol) in enumerate(subs):
                        nc.tensor.matmul(
                            ps[:, a:e],
                            lhsT=idsb[:],
                            rhs=dmt[:, dcol:dcol + (e - a)],
                            start=False, stop=(i == len(subs) - 1),
                            skip_group_check=True,
                        )
                nc.vector.tensor_reduce(
                    out=min_sb[:, t:t + 1], in_=ps[:, 0:W],
                    axis=mybir.AxisListType.X, op=mybir.AluOpType.min,
                )
                for s in range(NTYPES):
                    a, b_ = s * w, (s + 1) * w
                    ccol = t * NTYPES + s
                    if seg_engine(t, s) == "A":
                        sc = sc_act_pool.tile([128, WBMAX], f32, tag="sa")
                        nc.scalar.activation(
                            sc[:, a:b_], ps[:, a:b_],
                            mybir.ActivationFunctionType.Sign,
                            bias=bias36[:], scale=-1.0,
                            accum_out=cnt_sb[:, ccol:ccol + 1],
                        )
                    else:
                        sc = sc_dve_pool.tile([128, WBMAX], f32, tag="sd")
                        nc.vector.tensor_scalar(
                            out=sc[:, a:b_], in0=ps[:, a:b_],
                            scalar1=RCUT2, scalar2=None,
                            op0=mybir.AluOpType.is_lt,
                            op1=mybir.AluOpType.add,
                            accum_out=cnt_sb[:, ccol:ccol + 1],
                        )
                nc.sync.dma_start(out=out_min[:, t:t + 1],
                                  in_=min_sb[:, t:t + 1])
                nc.sync.dma_start(
                    out=out_cnt[:, t * NTYPES:(t + 1) * NTYPES],
                    in_=cnt_sb[:, t * NTYPES:(t + 1) * NTYPES])

            def body(_iv=None):
                for t in range(NQT):
                    do_slot(t, make_dm(t))

            if repeat == 1:
                body()
            else:
                with tc.For_i(0, repeat, 1) as iv:
                    body(iv)



    nc.compile()
    return nc


def _prep(coord, atype):
    """Host-side prep.

    Returns (in_maps, perms, tile_orders, wslots)."""
    c = np.asarray(coord, dtype=np.float32).reshape(NFRAMES, NLOC, 3)
    at = np.asarray(atype)

    perms, atsorted, csorted = [], [], []
    cand = [[None] * NTILES for _ in range(NFRAMES)]
    for f in range(NFRAMES):
        perm = _kd_perm(c[f])
        perms.append(perm)
        cs = c[f][perm]
        ats = at[f][perm]
        csorted.append(cs)
        atsorted.append(ats)
        cs64 = cs.astype(np.float64)
        for tg in range(NTILES):
            q = cs64[tg * 128:(tg + 1) * 128]
            lo, hi = q.min(0), q.max(0)
            d = np.maximum(np.maximum(lo - cs64, cs64 - hi), 0.0)
            idx = np.nonzero((d * d).sum(1) <= (RCUT + 0.01) ** 2)[0]
            # tile's own atoms first within each type (diagmask window)
            own = (idx >= tg * 128) & (idx < (tg + 1) * 128)
            bytype = []
            for tt in range(NTYPES):
                sel = idx[ats[idx] == tt]
                o = sel[(sel >= tg * 128) & (sel < (tg + 1) * 128)]
                rest = sel[(sel < tg * 128) | (sel >= (tg + 1) * 128)]
                bytype.append(np.concatenate([o, rest]))
            cand[f][tg] = bytype

    # processing order: per core, tiles sorted by type-max width descending
    tile_orders = []       # per core: list of global tile ids in slot order
    for core in range(NCORES):
        f, s = core // CPF, core % CPF
        tiles = list(range(s * NQT, (s + 1) * NQT))
        tiles.sort(key=lambda tg: -max(len(b) for b in cand[f][tg]))
        tile_orders.append(tiles)
    wslots = []
    for t in range(NQT):
        wmax = 0
        for core in range(NCORES):
            f = core // CPF
            tg = tile_orders[core][t]
            wmax = max(wmax, max(len(b) for b in cand[f][tg]))
        wslots.append(max(128, ((wmax + 31) // 32) * 32))
    Ws = [4 * w for w in wslots]
    col0 = np.concatenate([[0], np.cumsum(Ws)]).astype(int)
    WTOT = int(col0[-1])

    ident = np.eye(128, dtype=ml_dtypes.bfloat16)

    in_maps = []
    for core in range(NCORES):
        f, s = core // CPF, core % CPF
        cs64 = csorted[f].astype(np.float64) - CENTER
        qfeat_all, kfeat_all = _features(cs64)
        padf = _features(np.full((1, 3), PAD_COORD - CENTER))[1][:, 0]
        kfeat = np.empty((K, WTOT), ml_dtypes.bfloat16)
        selfpos = np.zeros((128, NQT), np.float32)
        qfeat = np.empty((K, QPC), ml_dtypes.bfloat16)
        for t in range(NQT):
            tg = tile_orders[core][t]
            w = wslots[t]
            qfeat[:, t * 128:(t + 1) * 128] = \
                qfeat_all[:, tg * 128:(tg + 1) * 128]
            for tt in range(NTYPES):
                ids = cand[f][tg][tt]
                n = len(ids)
                seg = int(col0[t]) + tt * w
                kfeat[:, seg:seg + n] = kfeat_all[:, ids]
                kfeat[:, seg + n:seg + w] = padf[:, None]
                # own atoms sit at positions 0..m-1 of this segment
                for j, a_ in enumerate(ids):
                    if tg * 128 <= a_ < (tg + 1) * 128:
                        p = int(a_) - tg * 128
                        selfpos[p, t] = tt * 128 + j
        in_maps.append({
            "qfeat": qfeat,
            "kfeat": kfeat,
            "selfpos": selfpos,
            "ident": ident,
        })
    return in_maps, perms, tile_orders, wslots


def _postprocess(results, perms, tile_orders, wslots, coord, atype):
    c = np.asarray(coord, dtype=np.float32).reshape(NFRAMES, NLOC, 3)
    min_rr2 = np.empty((NFRAMES, NLOC), np.float32)
    max_nnei = np.empty((NFRAMES, NTYPES), np.int64)
    nnei_max = np.zeros((NFRAMES, NTYPES), np.int64)
    mins_sorted = [np.empty(NLOC, np.float32) for _ in range(NFRAMES)]
    for core in range(NCORES):
        f, s = core // CPF, core % CPF
        r = results[core]
        mn = r["out_min"]                      # (128, NQT)
        cnt = r["out_cnt"].reshape(128, NQT, NTYPES)
        for t in range(NQT):
            tg = tile_orders[core][t]
            mins_sorted[f][tg * 128:(tg + 1) * 128] = mn[:, t]
            for tt in range(NTYPES):
                if seg_engine(t, tt) == "A":
                    lt = (cnt[:, t, tt] + wslots[t]) * 0.5
                else:
                    lt = cnt[:, t, tt]
                nnei_max[f, tt] = max(nnei_max[f, tt],
                                      int(np.round(lt.max())))
    for f in range(NFRAMES):
        ms = mins_sorted[f]
        bad = np.nonzero(ms >= RCUT2)[0]
        if len(bad):
            cs = c[f][perms[f]].astype(np.float32)
            for i in bad:
                d = cs - cs[i]
                rr = (d[:, 0] * d[:, 0] + d[:, 1] * d[:, 1]
                      + d[:, 2] * d[:, 2]).astype(np.float32)
                rr[i] = np.inf
                ms[i] = rr.min()
        min_rr2[f, perms[f]] = ms
        max_nnei[f] = nnei_max[f]
    return min_rr2, max_nnei.astype(np.int32)


def kernel(coord, atype):
    from concourse.bass_utils import run_bass_kernel_spmd

    in_maps, perms, tile_orders, wslots = _prep(coord, atype)
    key = tuple(wslots)
    if key not in _CACHE:
        _CACHE[key] = _build(wslots)
    nc = _CACHE[key]
    res = run_bass_kernel_spmd(nc, in_maps, list(range(NCORES)))
    return _postprocess(res.results, perms, tile_orders, wslots,
                        coord, atype)
